# revision 13
# baseline (speedup 1.0000x reference)
"""Trainium2 Bass kernel for GQA attention with RoPE (nn_Attention).

Reference (B=2, TQ=TKV=2048, D=2048, HQ=16, HKV=4, HD=128):
    q = Xq @ Wq; k = Xkv @ Wk; v = Xkv @ Wv
    q, k = rope(q, q_pos), rope(k, kv_pos)
    out = (causal_softmax(q k^T / sqrt(HD)) v) @ Wo   (kv head h//4 serves q head h)

Sharding: 8 cores = 2 batches x 4 query shards. Each core owns 8 interleaved
64-row query chunks (chunk i of core j is 4i + (j if i even else 3-j), which
balances the causal work exactly) and all 16 heads for those rows, so the
output projection needs no inter-core reduction. K/V projections are sharded
over the sequence (512 rows per core) and exchanged with AllGathers within
each batch group of 4 cores.

v2 restructure vs baseline:
  - K/V projections run first (kt-major so input DMA chunks pipeline), and
    the K/V exchange is split into 4 per-kv-head AllGather pieces that fire
    as soon as that head's data is bounced out. Attention for kv head h only
    depends on piece h, so the collectives hide under Q projection and
    earlier attention groups.
  - Emission order interleaves Q-projection head groups with attention
    kv-head groups to keep the PE dense (no HAM re-throttle) while pieces
    land.
  - Elementwise work is batched: exp activations cover two heads per
    instruction (ACT has a ~352-cycle fixed cost per op), PSUM tiles are
    multi-bank quads aligned so every matmul slice maps to one bank, RoPE /
    softmax-accumulation / mask ops run on wide strided APs.

Scores are computed transposed (S^T[kv, q]) so attention*V needs no
transposes. The SPMD NEFF is identical on all cores, so the causal block
schedule is the conservative core-independent one: kv block g (128 rows)
runs against query columns [64*(g//2) : 512]; only the first 64-col
sub-block's validity differs per core and is handled by a multiplicative
0/1 mask shipped as data. Softmax denominators accumulate on DVE in f32r;
normalization is folded into a PSUM->SBUF multiply of the context.
"""
import numpy as np
import ml_dtypes

B = 2
T = 2048
D = 2048
HQ = 16
HKV = 4
HD = 128
HALF = HD // 2
N_CORES = 8
QROWS = 512
KVSH = 512
SCALE = 1.0 / float(np.sqrt(HD))
MAX_TIMESCALE = 10000.0

# 8 chunks of 64 query rows per core; chunk i lives in [4i, 4i+3]
CHUNKS = [[4 * i + (j if i % 2 == 0 else 3 - j) for i in range(8)]
          for j in range(4)]

_CACHE = {}


def _build():
    import concourse.mybir as mybir
    import concourse.tile as tile
    from concourse import bacc

    bf = mybir.dt.bfloat16
    f32 = mybir.dt.float32
    f32r = mybir.dt.float32r

    nc = bacc.Bacc("TRN2", target_bir_lowering=False, debug=False,
                   num_devices=N_CORES)

    # activations/weights arrive pre-arranged so SBUF loads are contiguous:
    # [128 partition, 16 k-tiles * cols]
    xqT = nc.dram_tensor("xqT", [128, 16 * QROWS], bf, kind="ExternalInput").ap()
    xkvT = nc.dram_tensor("xkvT", [128, 16 * KVSH], bf, kind="ExternalInput").ap()
    wq = nc.dram_tensor("wq", [128, HQ * 16 * HD], bf, kind="ExternalInput").ap()
    wk = nc.dram_tensor("wk", [128, 16 * HKV * HD], bf, kind="ExternalInput").ap()
    wv = nc.dram_tensor("wv", [128, 16 * HKV * HD], bf, kind="ExternalInput").ap()
    wo = nc.dram_tensor("wo", [HQ * HD, D], bf, kind="ExternalInput").ap()
    # cos/sin shipped pre-tiled for batched rope: q 2-wide, kv 4-wide
    cosq2 = nc.dram_tensor("cosq2", [HD, 2 * QROWS], bf, kind="ExternalInput").ap()
    sinq2 = nc.dram_tensor("sinq2", [HD, 2 * QROWS], bf, kind="ExternalInput").ap()
    coskv4 = nc.dram_tensor("coskv4", [HD, 4 * KVSH], bf, kind="ExternalInput").ap()
    sinkv4 = nc.dram_tensor("sinkv4", [HD, 4 * KVSH], bf, kind="ExternalInput").ap()
    dmask = nc.dram_tensor("dmask", [16, 128, 256], bf, kind="ExternalInput").ap()
    selbc = nc.dram_tensor("selbc", [4, 4 * HD], f32, kind="ExternalInput").ap()
    out = nc.dram_tensor("out", [QROWS, D], f32, kind="ExternalOutput").ap()

    Exp = mybir.ActivationFunctionType.Exp
    PIECE = 2 * 65536  # K [128,512] + V [512,128] per piece, bf16 elems

    with tile.TileContext(nc) as tc:
        with tc.tile_pool(name="dram", bufs=1, space="DRAM") as dram, \
             tc.tile_pool(name="persist", bufs=1) as persist:

            # ---------------- persistent SBUF tiles ----------------
            # roped Q^T per group of 4 heads: [hd, 4*512]
            qt_sb = [persist.tile([HD, 4 * QROWS], bf, name=f"qtg{g}")
                     for g in range(4)]
            kt_sb = [persist.tile([HD, T], bf, name=f"ktg{h}") for h in range(HKV)]
            v_sb = [persist.tile([128, 16 * HD], bf, name=f"vg{h}") for h in range(HKV)]
            ctxn_sb = [persist.tile([HD, QROWS], bf, name=f"ctxn{h}") for h in range(HQ)]
            mask_sb = persist.tile([128, 16 * 128], bf, name="mask_sb")
            cq = persist.tile([HD, 2 * QROWS], bf, name="cq")
            sq = persist.tile([HD, 2 * QROWS], bf, name="sq")
            # final-sums lhsT: sel4r[qh] = [128, 4] f32r, only column qh ones
            sel4_f = [persist.tile([128, 4], f32, name=f"sel4f_{q}") for q in range(4)]
            sel4r = [persist.tile([128, 4], f32r, name=f"sel4r_{q}") for q in range(4)]
            # bcast lhsT: sel128r[qh] = [4, 128] f32r with only row qh ones
            sel128_f = persist.tile([4, 4 * HD], f32, name="sel128_f")
            sel128r_all = persist.tile([4, 4 * HD], f32r, name="sel128r_all")
            sel128r = [sel128r_all[:, q * HD:(q + 1) * HD] for q in range(4)]

            nc.sync.dma_start(cq[:], cosq2)
            nc.sync.dma_start(sq[:], sinq2)
            nc.sync.dma_start(mask_sb.rearrange("p (g c) -> p g c", g=16),
                              dmask.rearrange("g p c -> p g c")[:, :, 0:128])
            nc.sync.dma_start(sel128_f[:], selbc)
            nc.vector.tensor_copy(sel128r_all[:], sel128_f[:])
            for q in range(4):
                nc.vector.memset(sel4_f[q][:], 0.0)
                nc.vector.memset(sel4_f[q][:, q:q + 1], 1.0)
                nc.vector.tensor_copy(sel4r[q][:], sel4_f[q][:])

            # bounce buffers: piece h = K^T head h [128,512] ++ V head h [512,128]
            kv_in = dram.tile([HKV * PIECE], bf, name="kv_in")
            kv_out = dram.tile([4 * HKV * PIECE], bf, name="kv_out")

            # ---------------- phase A: K/V projections + AG pieces ----------
            psA = tc.tile_pool(name="psA", bufs=2, space="PSUM")
            psA_cm = psA.__enter__()
            sbA = tc.tile_pool(name="sbA", bufs=1)
            sbA_cm = sbA.__enter__()

            ckv = sbA_cm.tile([HD, 4 * KVSH], bf, name="ckv")
            skv = sbA_cm.tile([HD, 4 * KVSH], bf, name="skv")
            nc.sync.dma_start(ckv[:], coskv4)
            nc.sync.dma_start(skv[:], sinkv4)
            wv_sb = sbA_cm.tile([128, 16 * HKV * HD], bf, name="wv_sb")
            nc.sync.dma_start(wv_sb[:], wv)
            wk_sb = sbA_cm.tile([128, 16 * HKV * HD], bf, name="wk_sb")
            nc.sync.dma_start(wk_sb[:], wk)
            xkv_sb = sbA_cm.tile([128, 16 * KVSH], bf, name="xkv_sb")
            for ch in range(4):
                nc.sync.dma_start(xkv_sb[:, ch * 4 * KVSH:(ch + 1) * 4 * KVSH],
                                  xkvT[:, ch * 4 * KVSH:(ch + 1) * 4 * KVSH])

            # K^T quad [hd, (h,512 kv)]; kt-major so DMA chunks pipeline
            kq = psA_cm.tile([HD, 4 * KVSH], f32, tag="quad", name="kq")
            for kt in range(16):
                for h in range(HKV):
                    nc.tensor.matmul(
                        kq[:, h * KVSH:(h + 1) * KVSH],
                        wk_sb[:, kt * 512 + h * HD:kt * 512 + (h + 1) * HD],
                        xkv_sb[:, kt * KVSH:(kt + 1) * KVSH],
                        start=(kt == 0), stop=(kt == 15))
            kraw = sbA_cm.tile([HD, 4 * KVSH], bf, name="kraw")
            nc.scalar.copy(kraw[:], kq[:])
            # batched rope over all 4 kv heads
            ktr = sbA_cm.tile([HD, 4 * KVSH], bf, name="ktr")
            kt1 = sbA_cm.tile([HALF, 4 * KVSH], bf, name="kt1")
            kt2 = sbA_cm.tile([HALF, 4 * KVSH], bf, name="kt2")
            nc.vector.tensor_mul(kt1[:], kraw[0:HALF, :], ckv[0:HALF, :])
            nc.vector.tensor_mul(kt2[:], kraw[HALF:HD, :], skv[HALF:HD, :])
            nc.vector.tensor_sub(ktr[0:HALF, :], kt1[:], kt2[:])
            kt3 = sbA_cm.tile([HALF, 4 * KVSH], bf, name="kt3")
            kt4 = sbA_cm.tile([HALF, 4 * KVSH], bf, name="kt4")
            nc.vector.tensor_mul(kt3[:], kraw[0:HALF, :], skv[0:HALF, :])
            nc.vector.tensor_mul(kt4[:], kraw[HALF:HD, :], ckv[HALF:HD, :])
            nc.vector.tensor_add(ktr[HALF:HD, :], kt3[:], kt4[:])

            # V quad [128 kv-in-block, (b, h, hd)]
            vq = psA_cm.tile([128, 4 * 512], f32, tag="quad", name="vq")
            for kt in range(16):
                for b in range(4):
                    nc.tensor.matmul(
                        vq[:, b * 512:(b + 1) * 512],
                        xkv_sb[:, kt * KVSH + b * 128:kt * KVSH + (b + 1) * 128],
                        wv_sb[:, kt * 512:(kt + 1) * 512],
                        start=(kt == 0), stop=(kt == 15))
            vsh = sbA_cm.tile([128, 4 * 512], bf, name="vsh")
            nc.scalar.copy(vsh[:], vq[:])

            # bounce out + AG per piece (K part then V part)
            for h in range(HKV):
                base = h * PIECE
                nc.sync.dma_start(
                    kv_in[base:base + 65536].rearrange("(p c) -> p c", p=HD),
                    ktr[:, h * KVSH:(h + 1) * KVSH])
                # V head h: cols b*512 + h*128 + (0:128) -> [(b p) hd]
                nc.sync.dma_start(
                    kv_in[base + 65536:base + 2 * 65536].rearrange(
                        "(b p c) -> p b c", b=4, p=128),
                    vsh.rearrange("p (b c) -> p b c", b=4)[:, :, h * HD:(h + 1) * HD])
                nc.gpsimd.collective_compute(
                    "AllGather", mybir.AluOpType.bypass,
                    replica_groups=[[0, 1, 2, 3], [4, 5, 6, 7]],
                    ins=[kv_in[base:base + PIECE].opt()],
                    outs=[kv_out[4 * base:4 * base + 4 * PIECE].opt()])

            # phase A SBUF/PSUM frees here; wo tiles can reuse those
            # addresses so the 8MB Wo load overlaps attention
            sbA.__exit__(None, None, None)
            psA.__exit__(None, None, None)

            # ---------------- phase B: Q proj interleaved with attention ----
            sbQ = tc.tile_pool(name="sbQ", bufs=1)
            sbQ_cm = sbQ.__enter__()
            wq_pool = tc.tile_pool(name="wq_pool", bufs=2)
            wq_cm = wq_pool.__enter__()
            rtmp = tc.tile_pool(name="rtmp", bufs=2)
            rtmp_cm = rtmp.__enter__()
            # shared [128,1024] PSUM pool: Q-proj pairs + score pairs
            ps2 = tc.tile_pool(name="ps2", bufs=2, space="PSUM")
            ps2_cm = ps2.__enter__()
            ctx_ps = tc.tile_pool(name="ctx_ps", bufs=1, space="PSUM")
            ctx_cm = ctx_ps.__enter__()
            epool = tc.tile_pool(name="epool", bufs=4)
            e_cm = epool.__enter__()
            apool = tc.tile_pool(name="apool", bufs=2)
            a_cm = apool.__enter__()
            npool = tc.tile_pool(name="npool", bufs=2)
            n_cm = npool.__enter__()

            xq_sb = sbQ_cm.tile([128, 16 * QROWS], bf, name="xq_sb")
            for ch in range(4):
                nc.sync.dma_start(xq_sb[:, ch * 4 * QROWS:(ch + 1) * 4 * QROWS],
                                  xqT[:, ch * 4 * QROWS:(ch + 1) * 4 * QROWS])

            # output-projection weights prefetch (overlaps attention)
            wo_pool = tc.tile_pool(name="wo_pool", bufs=16, side="right")
            wo_cm = wo_pool.__enter__()
            wo_sb = []
            for h in range(HQ):
                t = wo_cm.tile([HD, D], bf, tag="wo", name=f"wosb{h}")
                nc.sync.dma_start(t[:], wo[h * HD:(h + 1) * HD, :])
                wo_sb.append(t)

            def qproj_group(grp):
                # two pairs of heads -> qt_sb[grp] [hd, 4*512]
                for pr in range(2):
                    h0 = 4 * grp + 2 * pr
                    wq_sbs = []
                    for hl in range(2):
                        h = h0 + hl
                        t = wq_cm.tile([128, 16 * HD], bf, tag="wq", name=f"wqsb{h}")
                        nc.sync.dma_start(t[:], wq[:, h * 2048:(h + 1) * 2048])
                        wq_sbs.append(t)
                    ps = ps2_cm.tile([HD, 2 * QROWS], f32, tag="p2", name=f"qps{grp}_{pr}")
                    for kt in range(16):
                        for hl in range(2):
                            nc.tensor.matmul(
                                ps[:, hl * QROWS:(hl + 1) * QROWS],
                                wq_sbs[hl][:, kt * HD:(kt + 1) * HD],
                                xq_sb[:, kt * QROWS:(kt + 1) * QROWS],
                                start=(kt == 0), stop=(kt == 15))
                    qraw = rtmp_cm.tile([HD, 2 * QROWS], bf, tag="qraw",
                                        name=f"qraw{grp}_{pr}")
                    nc.scalar.copy(qraw[:], ps[:])
                    qt = qt_sb[grp][:, pr * 2 * QROWS:(pr + 1) * 2 * QROWS]
                    t1 = rtmp_cm.tile([HALF, 2 * QROWS], bf, tag="t1", name=f"qt1_{grp}{pr}")
                    t2 = rtmp_cm.tile([HALF, 2 * QROWS], bf, tag="t2", name=f"qt2_{grp}{pr}")
                    nc.vector.tensor_mul(t1[:], qraw[0:HALF, :], cq[0:HALF, :])
                    nc.vector.tensor_mul(t2[:], qraw[HALF:HD, :], sq[HALF:HD, :])
                    nc.vector.tensor_sub(qt[0:HALF, :], t1[:], t2[:])
                    t3 = rtmp_cm.tile([HALF, 2 * QROWS], bf, tag="t1", name=f"qt3_{grp}{pr}")
                    t4 = rtmp_cm.tile([HALF, 2 * QROWS], bf, tag="t2", name=f"qt4_{grp}{pr}")
                    nc.vector.tensor_mul(t3[:], qraw[0:HALF, :], sq[0:HALF, :])
                    nc.vector.tensor_mul(t4[:], qraw[HALF:HD, :], cq[HALF:HD, :])
                    nc.vector.tensor_add(qt[HALF:HD, :], t3[:], t4[:])

            def attn_group(kvh):
                # unpack AG piece kvh
                base4 = 4 * kvh * PIECE
                for r in range(4):
                    rb = base4 + r * PIECE
                    nc.sync.dma_start(
                        kt_sb[kvh][:, r * 512:(r + 1) * 512],
                        kv_out[rb:rb + 65536].rearrange("(p c) -> p c", p=HD))
                    nc.sync.dma_start(
                        v_sb[kvh][:, r * 512:(r + 1) * 512].rearrange(
                            "p (b c) -> p b c", b=4),
                        kv_out[rb + 65536:rb + 2 * 65536].rearrange(
                            "(b p c) -> p b c", b=4, p=128))
                ctx = ctx_cm.tile([HD, 4 * QROWS], f32, tag="ctx", name=f"ctx{kvh}")
                acc = [a_cm.tile([128, 2 * QROWS], f32r, tag="acc",
                                 name=f"acc{kvh}_{pr}") for pr in range(2)]
                for g in range(16):
                    off = 64 * (g // 2)
                    ng = QROWS - off
                    ets = []
                    for pr in range(2):
                        sc = ps2_cm.tile([128, 2 * QROWS], f32, tag="p2",
                                         name=f"sc{kvh}_{g}_{pr}")
                        for hl in range(2):
                            qh = 2 * pr + hl
                            nc.tensor.matmul(
                                sc[:, hl * QROWS:hl * QROWS + ng],
                                kt_sb[kvh][:, g * 128:(g + 1) * 128],
                                qt_sb[kvh][:, qh * QROWS + off:(qh + 1) * QROWS],
                                start=True, stop=True)
                        et = e_cm.tile([128, 2 * QROWS], bf, tag="exp",
                                       name=f"et{kvh}_{g}_{pr}")
                        nc.scalar.activation(
                            et.rearrange("p (q c) -> p q c", q=2)[:, :, 0:ng],
                            sc.rearrange("p (q c) -> p q c", q=2)[:, :, 0:ng],
                            Exp, scale=SCALE)
                        nc.vector.tensor_mul(
                            et.rearrange("p (q c) -> p q c", q=2)[:, :, 0:64],
                            et.rearrange("p (q c) -> p q c", q=2)[:, :, 0:64],
                            mask_sb.rearrange("p (g q c) -> p g q c", g=16, q=2)
                            [:, g, :, :])
                        ets.append(et)
                    with nc.allow_low_precision(reason="f32r softmax sums"):
                        for pr in range(2):
                            if g == 0:
                                nc.vector.tensor_copy(acc[pr][:], ets[pr][:])
                            else:
                                nc.vector.tensor_add(
                                    acc[pr].rearrange("p (q c) -> p q c", q=2)
                                    [:, :, off:QROWS],
                                    acc[pr].rearrange("p (q c) -> p q c", q=2)
                                    [:, :, off:QROWS],
                                    ets[pr].rearrange("p (q c) -> p q c", q=2)
                                    [:, :, 0:ng])
                    for pr in range(2):
                        for hl in range(2):
                            qh = 2 * pr + hl
                            nc.tensor.matmul(
                                ctx[:, qh * QROWS + off:(qh + 1) * QROWS],
                                v_sb[kvh][:, g * 128:(g + 1) * 128],
                                ets[pr][:, hl * QROWS:hl * QROWS + ng],
                                start=(g == 0), stop=(g == 15),
                                skip_group_check=True)
                # tail: denominators -> reciprocal -> broadcast -> normalize
                sums = ps2_cm.tile([4, QROWS], f32, tag="p2", name=f"sums{kvh}")
                for pr in range(2):
                    for hl in range(2):
                        qh = 2 * pr + hl
                        nc.tensor.matmul(
                            sums[:], sel4r[qh][:],
                            acc[pr][:, hl * QROWS:(hl + 1) * QROWS],
                            start=(qh == 0), stop=(qh == 3),
                            skip_group_check=True)
                recip = n_cm.tile([4, QROWS], f32r, tag="recip", name=f"recip{kvh}")
                with nc.allow_low_precision(reason="f32r softmax denominators"):
                    nc.vector.reciprocal(recip[:], sums[:])
                for pr in range(2):
                    bps = ps2_cm.tile([HD, 2 * QROWS], f32, tag="p2",
                                      name=f"bps{kvh}_{pr}")
                    for hl in range(2):
                        qh = 2 * pr + hl
                        nc.tensor.matmul(bps[:, hl * QROWS:(hl + 1) * QROWS],
                                         sel128r[qh], recip[:],
                                         start=True, stop=True)
                    bsb = n_cm.tile([HD, 2 * QROWS], f32, tag="bsb",
                                    name=f"bsb{kvh}_{pr}")
                    nc.scalar.copy(bsb[:], bps[:])
                    for hl in range(2):
                        qh = 2 * pr + hl
                        h = kvh * 4 + qh
                        nc.vector.tensor_mul(
                            ctxn_sb[h][:],
                            ctx[:, qh * QROWS:(qh + 1) * QROWS],
                            bsb[:, hl * QROWS:(hl + 1) * QROWS])

            # buffer one extra Q group ahead of each attention group so the
            # PE has ready work while AG pieces are still in flight
            qproj_group(0)
            qproj_group(1)
            attn_group(0)
            qproj_group(2)
            attn_group(1)
            qproj_group(3)
            attn_group(2)
            attn_group(3)

            for pool in [npool, apool, epool, ctx_ps, ps2,
                         rtmp, wq_pool, sbQ]:
                pool.__exit__(None, None, None)

            # ---------------- phase C: output projection ----------------
            with tc.tile_pool(name="out_ps", bufs=2, space="PSUM") as out_ps, \
                 tc.tile_pool(name="osb_pool", bufs=2) as osb_pool:
                for c in range(4):
                    ps = out_ps.tile([128, D], f32, tag="ops", name=f"ops{c}")
                    for sl in range(4):
                        for h in range(HQ):
                            nc.tensor.matmul(
                                ps[:, sl * 512:(sl + 1) * 512],
                                ctxn_sb[h][:, c * 128:(c + 1) * 128],
                                wo_sb[h][:, sl * 512:(sl + 1) * 512],
                                start=(h == 0), stop=(h == HQ - 1))
                    osb = osb_pool.tile([128, D], f32, tag="osb", name=f"osb{c}")
                    nc.scalar.copy(osb[:], ps[:])
                    nc.sync.dma_start(out[c * 128:(c + 1) * 128, :], osb[:])
            wo_pool.__exit__(None, None, None)

    nc.compile()
    return nc


def _prep_core_inputs(c, Xq, Xkv, wq2, wk2, wv2, wo2, q_positions, kv_positions):
    bfl = ml_dtypes.bfloat16
    b, j = divmod(c, 4)
    chunks = CHUNKS[j]
    qrows = np.concatenate([np.arange(64 * ch, 64 * ch + 64) for ch in chunks])
    kvrows = np.arange(512 * j, 512 * j + 512)

    inv_freq = 1.0 / (MAX_TIMESCALE **
                      (2.0 * np.arange(HALF, dtype=np.float32) / HD))
    pq = q_positions[b][qrows].astype(np.float32)
    pk = kv_positions[b][kvrows].astype(np.float32)
    fq = inv_freq[:, None] * pq[None, :]
    fk = inv_freq[:, None] * pk[None, :]

    # validity mask for the first 64-col sub-block of each kv block:
    # chunk i0 = g//2, columns are rows 64*c0..64*c0+63, valid iff kv <= q
    dm = np.zeros((16, 128, 64), dtype=np.float32)
    for g in range(16):
        c0 = chunks[g // 2]
        kv_idx = 128 * g + np.arange(128)[:, None]
        q_idx = 64 * c0 + np.arange(64)[None, :]
        dm[g] = (kv_idx <= q_idx).astype(np.float32)
    dm = np.tile(dm[:, :, None, :], (1, 1, 4, 1)).reshape(16, 128, 256)

    cosq = np.concatenate([np.cos(fq)] * 2, axis=0)   # [128, 512]
    sinq = np.concatenate([np.sin(fq)] * 2, axis=0)
    coskv = np.concatenate([np.cos(fk)] * 2, axis=0)
    sinkv = np.concatenate([np.sin(fk)] * 2, axis=0)

    xq_dev = np.ascontiguousarray(
        Xq[b][qrows, :].T.reshape(16, 128, QROWS).transpose(1, 0, 2)
        .reshape(128, 16 * QROWS))
    xkv_dev = np.ascontiguousarray(
        Xkv[b][kvrows, :].T.reshape(16, 128, KVSH).transpose(1, 0, 2)
        .reshape(128, 16 * KVSH))
    return dict(
        xqT=xq_dev.astype(bfl),
        xkvT=xkv_dev.astype(bfl),
        wq=wq2, wk=wk2, wv=wv2, wo=wo2,
        cosq2=np.tile(cosq, (1, 2)).astype(bfl),
        sinq2=np.tile(sinq, (1, 2)).astype(bfl),
        coskv4=np.tile(coskv, (1, 4)).astype(bfl),
        sinkv4=np.tile(sinkv, (1, 4)).astype(bfl),
        dmask=dm.astype(bfl),
        selbc=_selbc(),
    )


def _selbc():
    s = np.zeros((4, 4 * HD), dtype=np.float32)
    for q in range(4):
        s[q, q * HD:(q + 1) * HD] = 1.0
    return s


def kernel(Xq, Xkv, Wq, Wk, Wv, Wo, q_positions, kv_positions):
    from concourse import bass_utils

    Xq = np.asarray(Xq, dtype=np.float32)
    Xkv = np.asarray(Xkv, dtype=np.float32)
    Wq = np.asarray(Wq, dtype=np.float32)
    Wk = np.asarray(Wk, dtype=np.float32)
    Wv = np.asarray(Wv, dtype=np.float32)
    Wo = np.asarray(Wo, dtype=np.float32)
    q_positions = np.asarray(q_positions)
    kv_positions = np.asarray(kv_positions)

    if "nc" not in _CACHE:
        _CACHE["nc"] = _build()
    nc = _CACHE["nc"]

    bfl = ml_dtypes.bfloat16
    # wq: [128 p, h*16kt*128] so each head's lhsT block is contiguous
    wq2 = np.ascontiguousarray(
        Wq.reshape(16, 128, HQ, HD).transpose(1, 2, 0, 3)
        .reshape(128, HQ * 16 * HD)).astype(bfl)
    wk2 = np.ascontiguousarray(
        Wk.reshape(16, 128, HKV * HD).transpose(1, 0, 2)
        .reshape(128, 16 * HKV * HD)).astype(bfl)
    wv2 = np.ascontiguousarray(
        Wv.reshape(16, 128, HKV * HD).transpose(1, 0, 2)
        .reshape(128, 16 * HKV * HD)).astype(bfl)
    wo2 = np.ascontiguousarray(Wo.reshape(HQ * HD, D)).astype(bfl)

    in_maps = [_prep_core_inputs(c, Xq, Xkv, wq2, wk2, wv2, wo2,
                                 q_positions, kv_positions)
               for c in range(N_CORES)]

    res = bass_utils.run_bass_kernel_spmd(
        nc, in_maps, core_ids=list(range(N_CORES)),
        **_CACHE.get("run_kwargs", {}))
    _CACHE["last_results"] = res

    out = np.empty((B, T, D), dtype=np.float32)
    for c in range(N_CORES):
        b, j = divmod(c, 4)
        core_out = res.results[c]["out"]
        for i, ch in enumerate(CHUNKS[j]):
            out[b, 64 * ch:64 * ch + 64, :] = core_out[64 * i:64 * i + 64, :]
    return out
